# revision 8
# baseline (speedup 1.0000x reference)
"""Trainium2 Bass kernel for relative-position attention (nn_Attention).

Reference computation (B=16, C=128, H=W=32, HEADS=4, d=32, N=1024):
    qkv  = W_qkv @ x                          (1x1 conv, per-pixel matmul)
    S    = scale * (q^T k + q^T r)            where r = rw + rh  (broadcast)
         = scale * q^T (k + r)                <- position term folds into k
    P    = softmax(S, axis=-1)
    out  = P @ v^T
Sharding: data-parallel over batch, 2 batches per core on 8 cores.

The kernel is ScalarE-bound: exp(S) is 64 x [128,1024] tiles per core
and exp only exists on ACT (1 elem/cycle/lane @1.2GHz) => ~69us floor.
Everything else is scheduled to hide under a continuous exp stream.

Measured facts driving the design (microbenches on this part):
  - K=32 matmuls never warm the PE clock gate (495ns/mm at N=512
    forever), but FOUR row-tiled K=32 matmuls at tile_position
    (0/32/64/96, 0) run concurrently: 457ns per group of four. All 4
    heads' S chunks advance together per (jc, nf) step.
  - S psum packs head pairs: psT01[:, 0:512]=h0, [:, 512:]=h1
    (concurrent row-tiled matmuls land in different PSUM banks), so exp
    stays [128,1024] granularity: 2 exps per step, 64 total.
  - O accumulators pack pairs: rows 0:33 = even head (row 32 = Z via
    the ones column in vt), rows 64:97 = odd head, via col-tiled
    concurrent matmuls at tile_position (0,0)/(0,64).
  - Steps iterate nf-major (all j-chunks of one i-half first), so O
    groups and normalizes for the first i-half complete mid-batch
    instead of all landing in the kernel tail.
  - The next batch's q/k/v projections are hoisted into the current
    batch's step stream (steps 12/14) so the batch boundary costs no
    exp-stream gap. Pool-slot parity (2 tag-"s" allocations per step)
    is preserved by allocating projections in pairs.
  - PSUM: psS 2x[128,1024] (4 banks) + psO ring of 4x[128,512]
    (4 banks, also carries the v^T projection psum) = 8 banks exactly.
  - normalize per (head, i-half): DVE reciprocal reads the Z row
    directly from PSUM, DMA bounce through DRAM gives the
    partition-broadcast, DVE mul + per-slice output DMA. ScalarE does
    ONLY exp.
  - all matmul operands bf16 (host converts x/W), rel err ~6e-3 vs the
    2e-2 gate.
"""

import numpy as np

B, C, H, W = 16, 128, 32, 32
HEADS = 4
D = C // HEADS          # 32
N = H * W               # 1024
SCALE = float(D) ** -0.5
NCORES = 8
BPC = B // NCORES       # batches per core
DELAY = 2               # O-matmul deferral in (nf, jc) steps


def _build_kernel(nc, tc, tile, mybir, x_ap, wT_ap, rw_ap, rh_ap, out_ap):
    import concourse.bass as bass

    f32 = mybir.dt.float32
    bf16 = mybir.dt.bfloat16

    const = tc.alloc_tile_pool(name="const", bufs=1)
    sb = tc.alloc_tile_pool(name="sb", bufs=2)
    epool = tc.alloc_tile_pool(name="epool", bufs=12)
    psS = tc.alloc_tile_pool(name="psS", bufs=2, space="PSUM")
    psO = tc.alloc_tile_pool(name="psO", bufs=4, space="PSUM")
    dscratch = tc.alloc_tile_pool(name="dscratch", bufs=8, space="DRAM")

    # --- constants / replicated inputs ---
    x0_s = sb.tile([128, N], bf16, tag="x", name="x0_s")
    for nf in range(2):
        nc.sync.dma_start(out=x0_s[:, nf * 512:(nf + 1) * 512],
                          in_=x_ap[0, :, nf * 512:(nf + 1) * 512])
    w_s = const.tile([128, 3 * C], bf16)
    nc.sync.dma_start(out=w_s[:], in_=wT_ap[:])
    rw_s = const.tile([128, W], f32)
    nc.sync.dma_start(out=rw_s[:], in_=rw_ap[:])
    rh_s = const.tile([128, H], f32)
    nc.sync.dma_start(out=rh_s[:], in_=rh_ap[:])

    # persistent vt tiles ([v^T | 1] per j-chunk), ones column preset once
    # on the Pool engine (GpSimd) which is otherwise idle
    vt1 = [[const.tile([128, HEADS, D + 1], bf16, name=f"vt{b}_{jc}")
            for jc in range(8)] for b in range(BPC)]
    for b in range(BPC):
        for jc in range(8):
            nc.gpsimd.memset(vt1[b][jc][:, :, D:D + 1], 1.0)

    # r[p, y*W + x] = rw[p, x] + rh[p, y] in one DVE op via step-0 free dims
    r_s = const.tile([128, N], f32)
    rw_b = bass.AP(tensor=rw_s.tensor, offset=rw_s.offset,
                   ap=[list(rw_s.ap[0]), [0, H], list(rw_s.ap[1])])
    rh_b = bass.AP(tensor=rh_s.tensor, offset=rh_s.offset,
                   ap=[list(rh_s.ap[0]), list(rh_s.ap[1]), [0, W]])
    nc.vector.tensor_add(
        out=r_s[:].rearrange("p (y x) -> p y x", y=H), in0=rh_b, in1=rw_b
    )

    x_tiles = {0: x0_s}
    q_tiles, kp_tiles, psv_tiles = {}, {}, {}

    def load_x(b):
        if b in x_tiles:
            return x_tiles[b]
        x_s = sb.tile([128, N], bf16, tag="x", name=f"x{b}_s")
        for nf in range(2):
            sl = slice(nf * 512, (nf + 1) * 512)
            nc.sync.dma_start(out=x_s[:, sl], in_=x_ap[b, :, sl])
        x_tiles[b] = x_s
        return x_s

    def project_qk(b):
        """q/k projection matmuls + DVE drains (2 tag-'s' allocations,
        preserving the step parity of the psS ring)."""
        x_s = load_x(b)
        q_all = sb.tile([128, N], bf16, tag="q", name=f"q{b}")
        kp_all = sb.tile([128, N], bf16, tag="kp", name=f"kp{b}")
        for m in (0, 1):
            ps = psS.tile([128, N], f32, tag="s", name=f"ps_qk{b}_{m}")
            for nf in range(2):
                nc.tensor.matmul(
                    ps[:, nf * 512:(nf + 1) * 512],
                    lhsT=w_s[:, m * 128:(m + 1) * 128],
                    rhs=x_s[:, nf * 512:(nf + 1) * 512],
                    start=True, stop=True,
                )
            # drain in [64,N] halves so head-pair 01's S matmuls can start
            # after only half the DVE work
            for half in range(2):
                rows = slice(64 * half, 64 * half + 64)
                if m == 0:
                    # 1/sqrt(d) scale folded into W_qkv's q rows on host
                    nc.vector.tensor_copy(out=q_all[rows, :], in_=ps[rows, :])
                else:
                    nc.vector.tensor_add(out=kp_all[rows, :], in0=ps[rows, :],
                                         in1=r_s[rows, :])
        q_tiles[b], kp_tiles[b] = q_all, kp_all

    def project_v(b):
        """v^T directly: psv[:, jc*128:+128] = x_chunk^T @ W_v = [128n, (h d)]
        (2 tag-'o' allocations in the psO ring)."""
        x_s = load_x(b)
        psv = [psO.tile([128, 512], f32, tag="o", name=f"psv{b}_{i}")
               for i in range(2)]
        for jc in range(8):
            nc.tensor.matmul(
                psv[jc // 4][:, (jc % 4) * 128:(jc % 4 + 1) * 128],
                lhsT=x_s[:, jc * 128:(jc + 1) * 128],
                rhs=w_s[:, 2 * C:3 * C],
                start=True, stop=True,
            )
        psv_tiles[b] = psv

    def vt_copy(b, jc):
        psv = psv_tiles[b]
        nc.vector.tensor_copy(
            out=vt1[b][jc][:, :, 0:D],
            in_=psv[jc // 4][:, (jc % 4) * 128:(jc % 4 + 1) * 128].rearrange(
                "p (h d) -> p h d", h=HEADS),
        )

    o_queue = []      # deferred O-matmul thunks
    pending = []      # deferred normalize muls (absorb DMA broadcast latency)

    def norm_chain(b, pair, k, nf, ps_o, out_s):
        """Reciprocal of the Z row (read straight from PSUM), DRAM-bounce
        partition-broadcast, then a deferred out = O * (1/Z) multiply."""
        h = 2 * pair + k
        rz = sb.tile([1, 512], f32, tag="rz", name=f"rz{b}_{h}_{nf}")
        nc.vector.reciprocal(out=rz[:], in_=ps_o[64 * k + D:64 * k + D + 1, :])
        r_d = dscratch.tile([1, 512], f32, tag="rd", name=f"rd{b}_{h}_{nf}")
        nc.sync.dma_start(out=r_d[:], in_=rz[:])
        rb = sb.tile([D, 512], f32, tag="rb", name=f"rb{b}_{h}_{nf}")
        nc.sync.dma_start(out=rb[:], in_=r_d[0, :].partition_broadcast(D))

        def norm_tail():
            nc.vector.tensor_mul(
                out=out_s[h * D:(h + 1) * D, nf * 512:(nf + 1) * 512],
                in0=ps_o[64 * k:64 * k + D, :], in1=rb[:],
            )
            nc.sync.dma_start(
                out=out_ap[b, h * D:(h + 1) * D, nf * 512:(nf + 1) * 512],
                in_=out_s[h * D:(h + 1) * D, nf * 512:(nf + 1) * 512],
            )

        pending.append(norm_tail)

    def o_step(b, nf, jc, e01, e23, ps_o01, ps_o23, out_s):
        for pair, (ps_o, e_t) in enumerate(((ps_o01, e01), (ps_o23, e23))):
            for k in range(2):
                h = 2 * pair + k
                nc.tensor.matmul(
                    ps_o[64 * k:64 * k + D + 1, :],
                    lhsT=vt1[b][jc][:, h, :],
                    rhs=e_t[:, k * 512:(k + 1) * 512],
                    start=(jc == 0), stop=(jc == 7),
                    tile_position=(0, 64 * k),
                    skip_group_check=True,
                )
        if jc == 7:
            for pair, ps_o in enumerate((ps_o01, ps_o23)):
                for k in range(2):
                    norm_chain(b, pair, k, nf, ps_o, out_s)

    # --- main pipeline ---
    project_qk(0)
    project_v(0)
    for b in range(BPC):
        q_all, kp_all = q_tiles[b], kp_tiles[b]
        out_s = sb.tile([128, N], f32, tag="out", name=f"out{b}")
        ps_o = {}
        for step in range(16):
            nf, jc = divmod(step, 8)
            if jc == 0:
                ps_o[nf] = (
                    psO.tile([128, 512], f32, tag="o", name=f"o01_{b}_{nf}"),
                    psO.tile([128, 512], f32, tag="o", name=f"o23_{b}_{nf}"),
                )
            psT01 = psS.tile([128, N], f32, tag="s", name=f"s01_{b}_{step}")
            psT23 = psS.tile([128, N], f32, tag="s", name=f"s23_{b}_{step}")
            for h in range(4):
                T = psT01 if h < 2 else psT23
                col = (h % 2) * 512
                nc.tensor.matmul(
                    T[:, col:col + 512],
                    lhsT=kp_all[32 * h:32 * h + 32, jc * 128:(jc + 1) * 128],
                    rhs=q_all[32 * h:32 * h + 32, nf * 512:(nf + 1) * 512],
                    start=True, stop=True,
                    tile_position=(32 * h, 0),
                )
            e01 = epool.tile([128, N], bf16, tag="e", name=f"e01_{b}_{step}")
            e23 = epool.tile([128, N], bf16, tag="e", name=f"e23_{b}_{step}")
            nc.scalar.activation(out=e01[:], in_=psT01[:],
                                 func=mybir.ActivationFunctionType.Exp)
            nc.scalar.activation(out=e23[:], in_=psT23[:],
                                 func=mybir.ActivationFunctionType.Exp)
            if step < 8:
                vt_copy(b, step)
            o_queue.append((b, nf, jc, e01, e23) + ps_o[nf] + (out_s,))
            if len(o_queue) > DELAY:
                o_step(*o_queue.pop(0))
            if pending:
                pending.pop(0)()
            if b + 1 < BPC:
                if step == 12:
                    project_qk(b + 1)
                elif step == 14:
                    project_v(b + 1)

    while o_queue:
        o_step(*o_queue.pop(0))
    while pending:
        pending.pop(0)()

    for p in (dscratch, psO, psS, epool, sb, const):
        p.release()


def build_nc():
    """Build the Bass module (shared by kernel() and test harnesses)."""
    import concourse.bacc as bacc
    import concourse.tile as tile
    from concourse import mybir

    f32 = mybir.dt.float32
    bf16 = mybir.dt.bfloat16
    nc = bacc.Bacc("TRN2", target_bir_lowering=False, debug=False,
                   num_devices=NCORES)
    x_ap = nc.dram_tensor("x", [BPC, C, N], bf16, kind="ExternalInput").ap()
    wT_ap = nc.dram_tensor("wT", [C, 3 * C], bf16, kind="ExternalInput").ap()
    rw_ap = nc.dram_tensor("rw2", [HEADS * D, W], f32, kind="ExternalInput").ap()
    rh_ap = nc.dram_tensor("rh2", [HEADS * D, H], f32, kind="ExternalInput").ap()
    out_ap = nc.dram_tensor("out", [BPC, C, N], f32, kind="ExternalOutput").ap()

    with tile.TileContext(nc) as tc:
        _build_kernel(nc, tc, tile, mybir, x_ap, wT_ap, rw_ap, rh_ap, out_ap)
    nc.compile()
    return nc


def _to_bf16(a):
    import ml_dtypes
    return np.asarray(a, np.float32).astype(ml_dtypes.bfloat16)


def make_in_maps(x, W_qkv, rw, rh):
    x_ = np.ascontiguousarray(np.asarray(x, np.float32).reshape(B, C, N))
    wT = np.ascontiguousarray(np.asarray(W_qkv, np.float32).T)
    wT[:, 0:C] *= SCALE    # fold the attention score scale into q projection
    x_bf = _to_bf16(x_)
    wT_bf = _to_bf16(wT)
    rw_ = np.ascontiguousarray(np.asarray(rw, np.float32).reshape(HEADS * D, W))
    rh_ = np.ascontiguousarray(np.asarray(rh, np.float32).reshape(HEADS * D, H))
    return [
        {"x": x_bf[i * BPC:(i + 1) * BPC], "wT": wT_bf, "rw2": rw_, "rh2": rh_}
        for i in range(NCORES)
    ]


def kernel(x, W_qkv, rw, rh):
    from concourse.bass_utils import run_bass_kernel_spmd

    nc = build_nc()
    in_maps = make_in_maps(x, W_qkv, rw, rh)
    res = None
    for attempt in range(3):
        try:
            res = run_bass_kernel_spmd(nc, in_maps, list(range(NCORES)))
            break
        except Exception:
            # transient device errors (e.g. NRT_EXEC_UNIT_UNRECOVERABLE after
            # an earlier crashed run) usually clear on retry
            if attempt == 2:
                raise
    out = np.concatenate([r["out"] for r in res.results], axis=0)
    return out.reshape(B, C, H, W).astype(np.float32)


# revision 15
# speedup vs baseline: 1.0326x; 1.0326x over previous
"""Trainium2 Bass kernel for relative-position attention (nn_Attention).

Reference computation (B=16, C=128, H=W=32, HEADS=4, d=32, N=1024):
    qkv  = W_qkv @ x                          (1x1 conv, per-pixel matmul)
    S    = scale * (q^T k + q^T r)            where r = rw + rh  (broadcast)
         = scale * q^T (k + r)                <- position term folds into k
    P    = softmax(S, axis=-1)
    out  = P @ v^T
Sharding: data-parallel over batch, 2 batches per core on 8 cores.

The kernel is ScalarE-bound: exp(S) is 64 x [128,1024] tiles per core
and exp only exists on ACT (1 elem/cycle/lane @1.2GHz) => ~69us floor.
Everything else is scheduled to hide under a continuous exp stream.

Measured facts driving the design (microbenches on this part):
  - K=32 matmuls never warm the PE clock gate (495ns/mm at N=512
    forever), but FOUR row-tiled K=32 matmuls at tile_position
    (0/32/64/96, 0) run concurrently: 457ns per group of four. All 4
    heads' S chunks advance together per (jc, nf) step.
  - S psum packs head pairs: psT01[:, 0:512]=h0, [:, 512:]=h1
    (concurrent row-tiled matmuls land in different PSUM banks), so exp
    stays [128,1024] granularity: 2 exps per step, 64 total.
  - O accumulators pack pairs: rows 0:33 = even head (row 32 = Z via
    the ones column in vt), rows 64:97 = odd head, via col-tiled
    concurrent matmuls at tile_position (0,0)/(0,64).
  - Steps iterate nf-major (all j-chunks of one i-half first), so O
    groups and normalizes for the first i-half complete mid-batch
    instead of all landing in the kernel tail.
  - The next batch's q/k/v projections are hoisted into the current
    batch's step stream (steps 12/14) so the batch boundary costs no
    exp-stream gap. Pool-slot parity (2 tag-"s" allocations per step)
    is preserved by allocating projections in pairs.
  - PSUM: psS 2x[128,1024] (4 banks) + psO ring of 4x[128,512]
    (4 banks, also carries the v^T projection psum) = 8 banks exactly.
  - normalize per (head, i-half): DVE reciprocal reads the Z row
    directly from PSUM, DMA bounce through DRAM gives the
    partition-broadcast, DVE mul + per-slice output DMA. ScalarE does
    ONLY exp.
  - all matmul operands bf16 (host converts x/W), rel err ~6e-3 vs the
    2e-2 gate.
"""

import numpy as np

B, C, H, W = 16, 128, 32, 32
HEADS = 4
D = C // HEADS          # 32
N = H * W               # 1024
SCALE = float(D) ** -0.5
NCORES = 8
BPC = B // NCORES       # batches per core
DELAY = 2               # O-matmul deferral in (nf, jc) steps


def _build_kernel(nc, tc, tile, mybir, x_ap, wT_ap, rw_ap, rh_ap, out_ap):
    import concourse.bass as bass

    f32 = mybir.dt.float32
    bf16 = mybir.dt.bfloat16

    const = tc.alloc_tile_pool(name="const", bufs=1)
    sb = tc.alloc_tile_pool(name="sb", bufs=2)
    epool = tc.alloc_tile_pool(name="epool", bufs=16)
    psS = tc.alloc_tile_pool(name="psS", bufs=2, space="PSUM")
    psO = tc.alloc_tile_pool(name="psO", bufs=4, space="PSUM")
    dscratch = tc.alloc_tile_pool(name="dscratch", bufs=8, space="DRAM")

    # --- constants / replicated inputs ---
    x0_s = sb.tile([128, N], bf16, tag="x", name="x0_s")
    for nf in range(2):
        nc.sync.dma_start(out=x0_s[:, nf * 512:(nf + 1) * 512],
                          in_=x_ap[0, :, nf * 512:(nf + 1) * 512])
    w_s = const.tile([128, 3 * C], bf16)
    nc.sync.dma_start(out=w_s[:], in_=wT_ap[:])
    rw_s = const.tile([128, W], f32)
    nc.sync.dma_start(out=rw_s[:], in_=rw_ap[:])
    rh_s = const.tile([128, H], f32)
    nc.sync.dma_start(out=rh_s[:], in_=rh_ap[:])

    # persistent vt tiles ([v^T | 1] per j-chunk), ones column preset once
    # on the Pool engine (GpSimd) which is otherwise idle
    vt1 = [[const.tile([128, HEADS, D + 1], bf16, name=f"vt{b}_{jc}")
            for jc in range(8)] for b in range(BPC)]
    for b in range(BPC):
        for jc in range(8):
            nc.gpsimd.memset(vt1[b][jc][:, :, D:D + 1], 1.0)

    # r[p, y*W + x] = rw[p, x] + rh[p, y] in one DVE op via step-0 free dims
    r_s = const.tile([128, N], f32)
    rw_b = bass.AP(tensor=rw_s.tensor, offset=rw_s.offset,
                   ap=[list(rw_s.ap[0]), [0, H], list(rw_s.ap[1])])
    rh_b = bass.AP(tensor=rh_s.tensor, offset=rh_s.offset,
                   ap=[list(rh_s.ap[0]), list(rh_s.ap[1]), [0, W]])
    nc.vector.tensor_add(
        out=r_s[:].rearrange("p (y x) -> p y x", y=H), in0=rh_b, in1=rw_b
    )

    x_tiles = {0: x0_s}
    q_tiles, kp_tiles, psv_tiles = {}, {}, {}

    def load_x(b):
        if b in x_tiles:
            return x_tiles[b]
        x_s = sb.tile([128, N], bf16, tag="x", name=f"x{b}_s")
        for nf in range(2):
            sl = slice(nf * 512, (nf + 1) * 512)
            nc.sync.dma_start(out=x_s[:, sl], in_=x_ap[b, :, sl])
        x_tiles[b] = x_s
        return x_s

    def project_qk(b):
        """q/k projection matmuls + DVE drains (2 tag-'s' allocations,
        preserving the step parity of the psS ring)."""
        x_s = load_x(b)
        q_all = sb.tile([128, N], bf16, tag="q", name=f"q{b}")
        kp_all = sb.tile([128, N], bf16, tag="kp", name=f"kp{b}")
        pss = []
        for m in (0, 1):
            ps = psS.tile([128, N], f32, tag="s", name=f"ps_qk{b}_{m}")
            pss.append(ps)
            for nf in range(2):
                nc.tensor.matmul(
                    ps[:, nf * 512:(nf + 1) * 512],
                    lhsT=w_s[:, m * 128:(m + 1) * 128],
                    rhs=x_s[:, nf * 512:(nf + 1) * 512],
                    start=True, stop=True,
                )
        # drain in free-dim halves (DVE cost scales with free size, not
        # partitions), q/kp interleaved: step 0's S matmuls need only
        # q cols 0:512 and kp cols 0:128 — ready after two DVE ops.
        # 1/sqrt(d) scale is folded into W_qkv's q rows on host.
        for half in range(2):
            cols = slice(512 * half, 512 * half + 512)
            nc.vector.tensor_copy(out=q_all[:, cols], in_=pss[0][:, cols])
            nc.vector.tensor_add(out=kp_all[:, cols], in0=pss[1][:, cols],
                                 in1=r_s[:, cols])
        q_tiles[b], kp_tiles[b] = q_all, kp_all

    def project_v(b):
        """v^T directly: psv[:, jc*128:+128] = x_chunk^T @ W_v = [128n, (h d)]
        (2 tag-'o' allocations in the psO ring)."""
        x_s = load_x(b)
        psv = [psO.tile([128, 512], f32, tag="o", name=f"psv{b}_{i}")
               for i in range(2)]
        for jc in range(8):
            nc.tensor.matmul(
                psv[jc // 4][:, (jc % 4) * 128:(jc % 4 + 1) * 128],
                lhsT=x_s[:, jc * 128:(jc + 1) * 128],
                rhs=w_s[:, 2 * C:3 * C],
                start=True, stop=True,
            )
        psv_tiles[b] = psv

    def vt_copy(b, jc):
        psv = psv_tiles[b]
        nc.vector.tensor_copy(
            out=vt1[b][jc][:, :, 0:D],
            in_=psv[jc // 4][:, (jc % 4) * 128:(jc % 4 + 1) * 128].rearrange(
                "p (h d) -> p h d", h=HEADS),
        )

    o_queue = []      # deferred O-matmul thunks
    pending = []      # deferred normalize muls (absorb DMA broadcast latency)
    gstep = [0]       # global step counter (for pending-pop readiness)

    def norm_chain(b, pair, k, nf, ps_o, out_s):
        """Reciprocal of the Z row (read straight from PSUM), DRAM-bounce
        partition-broadcast, then a deferred out = O * (1/Z) multiply."""
        h = 2 * pair + k
        rz = sb.tile([1, 512], f32, tag="rz", name=f"rz{b}_{h}_{nf}")
        nc.vector.reciprocal(out=rz[:], in_=ps_o[64 * k + D:64 * k + D + 1, :])
        r_d = dscratch.tile([1, 512], f32, tag="rd", name=f"rd{b}_{h}_{nf}")
        nc.sync.dma_start(out=r_d[:], in_=rz[:])
        rb = sb.tile([D, 512], f32, tag="rb", name=f"rb{b}_{h}_{nf}")
        nc.sync.dma_start(out=rb[:], in_=r_d[0, :].partition_broadcast(D))

        def norm_tail():
            nc.vector.tensor_mul(
                out=out_s[h * D:(h + 1) * D, nf * 512:(nf + 1) * 512],
                in0=ps_o[64 * k:64 * k + D, :], in1=rb[:],
            )
            nc.sync.dma_start(
                out=out_ap[b, h * D:(h + 1) * D, nf * 512:(nf + 1) * 512],
                in_=out_s[h * D:(h + 1) * D, nf * 512:(nf + 1) * 512],
            )

        # hold the mul back ~3 steps so the rb broadcast DMA (+~2us
        # completion-semaphore latency) lands first — a waiting mul would
        # head-of-line-block the in-order DVE queue
        pending.append((gstep[0] + 3, norm_tail))

    def o_step(b, nf, jc, e01, e23, ps_o01, ps_o23, out_s):
        for pair, (ps_o, e_t) in enumerate(((ps_o01, e01), (ps_o23, e23))):
            for k in range(2):
                h = 2 * pair + k
                nc.tensor.matmul(
                    ps_o[64 * k:64 * k + D + 1, :],
                    lhsT=vt1[b][jc][:, h, :],
                    rhs=e_t[:, k * 512:(k + 1) * 512],
                    start=(jc == 0), stop=(jc == 7),
                    tile_position=(0, 64 * k),
                    skip_group_check=True,
                )
        if jc == 7:
            for pair, ps_o in enumerate((ps_o01, ps_o23)):
                for k in range(2):
                    norm_chain(b, pair, k, nf, ps_o, out_s)

    # --- main pipeline ---
    project_qk(0)
    project_v(0)
    for b in range(BPC):
        q_all, kp_all = q_tiles[b], kp_tiles[b]
        out_s = sb.tile([128, N], f32, tag="out", name=f"out{b}")
        ps_o = {}
        for step in range(16):
            gstep[0] = 16 * b + step
            nf, jc = divmod(step, 8)
            if jc == 0:
                ps_o[nf] = (
                    psO.tile([128, 512], f32, tag="o", name=f"o01_{b}_{nf}"),
                    psO.tile([128, 512], f32, tag="o", name=f"o23_{b}_{nf}"),
                )
            psT01 = psS.tile([128, N], f32, tag="s", name=f"s01_{b}_{step}")
            psT23 = psS.tile([128, N], f32, tag="s", name=f"s23_{b}_{step}")
            for h in range(4):
                T = psT01 if h < 2 else psT23
                col = (h % 2) * 512
                nc.tensor.matmul(
                    T[:, col:col + 512],
                    lhsT=kp_all[32 * h:32 * h + 32, jc * 128:(jc + 1) * 128],
                    rhs=q_all[32 * h:32 * h + 32, nf * 512:(nf + 1) * 512],
                    start=True, stop=True,
                    tile_position=(32 * h, 0),
                )
            e01 = epool.tile([128, N], bf16, tag="e", name=f"e01_{b}_{step}")
            e23 = epool.tile([128, N], bf16, tag="e", name=f"e23_{b}_{step}")
            nc.scalar.activation(out=e01[:], in_=psT01[:],
                                 func=mybir.ActivationFunctionType.Exp)
            nc.scalar.activation(out=e23[:], in_=psT23[:],
                                 func=mybir.ActivationFunctionType.Exp)
            if step < 8:
                vt_copy(b, step)
            o_queue.append((b, nf, jc, e01, e23) + ps_o[nf] + (out_s,))
            if len(o_queue) > DELAY:
                o_step(*o_queue.pop(0))
            if pending and pending[0][0] <= gstep[0]:
                pending.pop(0)[1]()
            if b + 1 < BPC:
                if step == 12:
                    project_qk(b + 1)
                elif step == 14:
                    project_v(b + 1)

    while o_queue:
        gstep[0] += 1
        o_step(*o_queue.pop(0))
    while pending:
        pending.pop(0)[1]()

    for p in (dscratch, psO, psS, epool, sb, const):
        p.release()


def build_nc():
    """Build the Bass module (shared by kernel() and test harnesses)."""
    import concourse.bacc as bacc
    import concourse.tile as tile
    from concourse import mybir

    f32 = mybir.dt.float32
    bf16 = mybir.dt.bfloat16
    nc = bacc.Bacc("TRN2", target_bir_lowering=False, debug=False,
                   num_devices=NCORES)
    x_ap = nc.dram_tensor("x", [BPC, C, N], bf16, kind="ExternalInput").ap()
    wT_ap = nc.dram_tensor("wT", [C, 3 * C], bf16, kind="ExternalInput").ap()
    rw_ap = nc.dram_tensor("rw2", [HEADS * D, W], f32, kind="ExternalInput").ap()
    rh_ap = nc.dram_tensor("rh2", [HEADS * D, H], f32, kind="ExternalInput").ap()
    out_ap = nc.dram_tensor("out", [BPC, C, N], f32, kind="ExternalOutput").ap()

    with tile.TileContext(nc) as tc:
        _build_kernel(nc, tc, tile, mybir, x_ap, wT_ap, rw_ap, rh_ap, out_ap)
    nc.compile()
    return nc


def _to_bf16(a):
    import ml_dtypes
    return np.asarray(a, np.float32).astype(ml_dtypes.bfloat16)


def make_in_maps(x, W_qkv, rw, rh):
    x_ = np.ascontiguousarray(np.asarray(x, np.float32).reshape(B, C, N))
    wT = np.ascontiguousarray(np.asarray(W_qkv, np.float32).T)
    wT[:, 0:C] *= SCALE    # fold the attention score scale into q projection
    x_bf = _to_bf16(x_)
    wT_bf = _to_bf16(wT)
    rw_ = np.ascontiguousarray(np.asarray(rw, np.float32).reshape(HEADS * D, W))
    rh_ = np.ascontiguousarray(np.asarray(rh, np.float32).reshape(HEADS * D, H))
    return [
        {"x": x_bf[i * BPC:(i + 1) * BPC], "wT": wT_bf, "rw2": rw_, "rh2": rh_}
        for i in range(NCORES)
    ]


def kernel(x, W_qkv, rw, rh):
    from concourse.bass_utils import run_bass_kernel_spmd

    nc = build_nc()
    in_maps = make_in_maps(x, W_qkv, rw, rh)
    res = None
    for attempt in range(3):
        try:
            res = run_bass_kernel_spmd(nc, in_maps, list(range(NCORES)))
            break
        except Exception:
            # transient device errors (e.g. NRT_EXEC_UNIT_UNRECOVERABLE after
            # an earlier crashed run) usually clear on retry
            if attempt == 2:
                raise
    out = np.concatenate([r["out"] for r in res.results], axis=0)
    return out.reshape(B, C, H, W).astype(np.float32)
